# revision 16
# baseline (speedup 1.0000x reference)
"""Biaffine edge attention on 8 Trainium2 NeuronCores.

Math (per batch b):
    out[i,o] = head[i,:] @ U @ dep[o,:] + head[i,:]@wh + dep[o,:]@wd + b
with head/dep [S=2048, D=256], U [D,D], edge_W = [wh | wd] (each [D]).

Sharding: pure data-parallel over batch B=8 -> one batch per core,
U / edge_W / edge_b replicated. No collectives.

Per-core kernel:
    ATf[e,i] = sum_d U[d,e] * headT[d,i] + wd[e]      (dep-side rank-1 term
               rides the e-contraction for free)
    hs[i]    = sum_d head[i,d] * wh[d]  + b           (DVE mul+reduce)
    out[i,o] = sum_e ATf[e,i] * depT[e,o]  + hs[i]

exec_time ~= first_store_time + 16.8MB/~420GB/s + wrapup, so the whole
schedule optimizes when the first out bytes hit the store ring:
  * head is loaded in "16-consecutive-rows-per-partition" layout
    (partition p holds rows 16p..16p+15) -> 8KB contiguous DMA
    descriptors, 2 DMAs.  Out tile r then covers rows {16p + r}, an
    affine DRAM AP with 8KB-contiguous rows, and hs lines up as
    hs_col[p, r].  dep keeps the 128-row-block layout (its transpose
    must produce natural o-order for contiguous stores).
  * Load order (all triggers upfront, ACT HWDGE ring): consts, head
    half0, dep g0..g3, head half1.  Out chunk (r, oc) needs only dep
    group oc + atf(half r//8), so the first stores issue ~16us while
    dep g1..g3 are still loading.
  * Tiles r0..7 store as [128, 2x512] pair-chunks (oc-major, chasing
    dep groups); r8..15 as full [128,2048] rows.  Stores go on the sync
    (SP) HWDGE ring, independent of the load ring.
  * Transpose collect copies split DVE/ACT so neither serializes depT.
Matmuls run as float32r (1 cycle/row for moving dim >= 256).  FP32r
matmul inputs must be produced by a compute op, so matmul-feeding SBUF
tiles are float32r-typed and written by DVE/ACT copies, never by DMA.
"""

import numpy as np

import concourse.bass as bass
import concourse.tile as tile
from concourse import bacc, mybir
from concourse.bass_utils import run_bass_kernel_spmd

B, S, D = 8, 2048, 256
P = 128          # partitions
OC = 512         # matmul output free-dim chunk (one PSUM bank of fp32)
RPP = 16         # head rows per partition (out tile r covers rows 16p+r)
NI = S // P      # 16 out row-tiles
NO = S // OC     # 4 output column chunks
ND = D // P      # 2 contraction chunks
NG = 4           # dep load groups (128-row blocks x4 per group)
GB = 4           # row-blocks per dep group
HH = 2           # head halves
RPH = 8          # r-blocks per head half
F32 = mybir.dt.float32
F32R = mybir.dt.float32r

# packed const layout: eye | U0 | U1 | wh | wdT | b
C_EYE, C_U0, C_U1, C_WH, C_WDT, C_B = 0, P, P + D, P + 2 * D, P + 3 * D, P + 3 * D + ND
C_TOT = C_B + 1


def build_nc(reps=1):
    nc = bacc.Bacc("TRN2", target_bir_lowering=False, debug=False, num_devices=B)

    head_d = nc.dram_tensor("head", [S, D], F32, kind="ExternalInput")
    dep_d = nc.dram_tensor("dep", [S, D], F32, kind="ExternalInput")
    cst_d = nc.dram_tensor("cpack", [P, C_TOT], F32, kind="ExternalInput")
    out_d = nc.dram_tensor("out", [S, S], F32, kind="ExternalOutput")

    Ident = mybir.ActivationFunctionType.Identity

    with tile.TileContext(nc) as tc:
        with (
            tc.tile_pool(name="const", bufs=1) as cpool,
            tc.tile_pool(name="persist", bufs=1) as ppool,
            tc.tile_pool(name="ttrp", bufs=2) as ttrp,
            tc.tile_pool(name="pairbuf", bufs=4) as pairbuf,
            tc.tile_pool(name="outbuf", bufs=4) as outbuf,
            tc.tile_pool(name="ps_t", bufs=3, space=bass.MemorySpace.PSUM) as ps_t,
            tc.tile_pool(name="ps_mm", bufs=5, space=bass.MemorySpace.PSUM) as ps_mm,
        ):
            # ---- all loads issued upfront on the ACT HWDGE ring ----
            cst = cpool.tile([P, C_TOT], F32, name="cst", tag="cst")
            nc.scalar.dma_start(cst[:], cst_d[:])

            # trigger FIFO order on the ACT ring: cst, head h0, dep g0..g3,
            # head h1 -- head h0 early unlocks ATf for tiles r0..7; dep
            # groups next so chunk (r, oc) stores can chase them; head h1
            # last (its tiles store latest).
            head3 = head_d[0:S, :].rearrange("(p rr) d -> p rr d", rr=RPP)

            def load_head_half(hh):
                t = ppool.tile([P, RPH * D], F32, name=f"nath{hh}", tag=f"nath{hh}")
                nc.scalar.dma_start(
                    t[:].rearrange("p (r d) -> p r d", d=D),
                    head3[:, hh * RPH:(hh + 1) * RPH, :],
                )
                return t

            def load_dep_group(g):
                t = ppool.tile([P, GB * D], F32, name=f"natd{g}", tag=f"natd{g}")
                src = dep_d[g * GB * P:(g + 1) * GB * P, :]
                nc.scalar.dma_start(
                    t[:].rearrange("p (j d) -> p j d", d=D),
                    src.rearrange("(j p) d -> p j d", p=P),
                )
                return t

            nat_h = [None, None]
            nat_h[0] = load_head_half(0)
            nat_d = [load_dep_group(g) for g in range(NG)]
            nat_h[1] = load_head_half(1)

            # ---- f32r copies of U (DVE, right after cst lands) ----
            u_sb = []
            for dc in range(ND):
                u_t = cpool.tile([P, D], F32R, name=f"u{dc}", tag=f"u{dc}")
                nc.vector.tensor_copy(u_t[:], cst[:, C_U0 + dc * D:C_U0 + (dc + 1) * D])
                u_sb.append(u_t)
            eye = cst[:, C_EYE:C_EYE + P]

            # ---- persistent SBUF tensors ----
            headT = [ppool.tile([P, S], F32R, name=f"headT{dc}", tag=f"headT{dc}")
                     for dc in range(ND)]
            depT = [ppool.tile([P, S], F32R, name=f"depT{dc}", tag=f"depT{dc}")
                    for dc in range(ND)]
            atf = [ppool.tile([P, S], F32R, name=f"atf{eb}", tag=f"atf{eb}")
                   for eb in range(ND)]
            hs_colb = ppool.tile([P, NI], F32, name="hs_colb", tag="hs_colb")

            out3 = out_d[0:S, :].rearrange("(p rr) o -> p rr o", rr=RPP)

            def transpose_head_half(hh):
                # 16 PE transposes -> 4 [128,512] collect tiles; alternate
                # DVE/ACT on the collects.  headT columns land as (r, p),
                # r = global row-tile index = hh*8 + rloc.
                nat = nat_h[hh]
                for dc in range(ND):
                    for q in range(2):
                        pst = ps_t.tile([P, 4 * P], F32, name="pst", tag="pst")
                        for k in range(4):
                            rloc = q * 4 + k
                            nc.tensor.transpose(
                                pst[:, k * P:(k + 1) * P],
                                nat[:, rloc * D + dc * P: rloc * D + dc * P + P],
                                eye,
                            )
                        dst = headT[dc][:, (hh * RPH + q * 4) * P:
                                         (hh * RPH + q * 4 + 4) * P]
                        if (dc + q) % 2 == 0:
                            nc.vector.tensor_copy(dst, pst[:])
                        else:
                            nc.scalar.copy(dst, pst[:])

            def transpose_dep_group(g):
                # g0/g1 collects split DVE/ACT (critical path to the first
                # stores); g2/g3 go to ACT which has slack by then.
                nat = nat_d[g]
                for dc in range(ND):
                    pst = ps_t.tile([P, GB * P], F32, name="pst", tag="pst")
                    for j in range(GB):
                        nc.tensor.transpose(
                            pst[:, j * P:(j + 1) * P],
                            nat[:, j * D + dc * P: j * D + dc * P + P],
                            eye,
                        )
                    dst = depT[dc][:, g * GB * P:(g + 1) * GB * P]
                    if g < 2 and dc == 0:
                        nc.vector.tensor_copy(dst, pst[:])
                    else:
                        nc.scalar.copy(dst, pst[:])

            def hs_block(r):
                # hs_colb[p, r] = b + sum_d nat_h[p, r*D+d] * wh[d] -- the
                # mul runs on the otherwise-idle GpSimd; only the cheap
                # X-reduce (+bias) lands on DVE.
                hh, rloc = r // RPH, r % RPH
                ttr = ttrp.tile([P, D], F32, name="ttr", tag="ttr")
                nc.gpsimd.tensor_mul(
                    ttr[:], nat_h[hh][:, rloc * D:(rloc + 1) * D],
                    cst[:, C_WH:C_WH + D],
                )
                hsr = ttrp.tile([P, 1], F32, name="hsr", tag="hsr")
                nc.vector.reduce_sum(hsr[:], ttr[:], axis=mybir.AxisListType.X)
                nc.vector.tensor_scalar_add(
                    hs_colb[:, r:r + 1], hsr[:], cst[:, C_B:C_B + 1],
                )

            def at_half(hh):
                # ATf for this half's 2 quad-chunks (each 512 i-columns)
                for q in range(2):
                    c0 = (hh * 2 + q) * OC
                    for eb in range(ND):
                        pa = ps_mm.tile([P, OC], F32, name="psmm", tag="psmm")
                        for dc in range(ND):
                            nc.tensor.matmul(
                                pa[:],
                                u_sb[dc][:, eb * P:(eb + 1) * P],
                                headT[dc][:, c0:c0 + OC],
                                start=(dc == 0),
                                stop=(dc == ND - 1),
                            )
                        nc.scalar.activation(
                            atf[eb][:, c0:c0 + OC], pa[:], Ident,
                            bias=cst[:, C_WDT + eb:C_WDT + eb + 1],
                        )

            def mm_chunk(r, oc, dst, eng):
                po = ps_mm.tile([P, OC], F32, name="psmm", tag="psmm")
                for eb in range(ND):
                    nc.tensor.matmul(
                        po[:],
                        atf[eb][:, r * P:(r + 1) * P],
                        depT[eb][:, oc * OC:(oc + 1) * OC],
                        start=(eb == 0),
                        stop=(eb == ND - 1),
                    )
                if eng == 0:
                    nc.scalar.activation(dst, po[:], Ident, bias=hs_colb[:, r:r + 1])
                else:
                    nc.vector.tensor_scalar_add(dst, po[:], hs_colb[:, r:r + 1])

            def body():
                transpose_head_half(0)
                for r in range(RPH):
                    hs_block(r)
                transpose_dep_group(0)
                at_half(0)
                transpose_dep_group(1)
                # phase A: tiles r0..7 as [128, 2 rows x 1024 cols] mega-chunk
                # stores (4KB descriptors), oc-half-major chasing dep groups.
                # Single epilogue engine per store tile (alternating).
                for och in range(2):
                    if och == 1:
                        transpose_dep_group(2)
                        transpose_dep_group(3)
                    for pr in range(RPH // 2):
                        pt = pairbuf.tile([P, 2 * 2 * OC], F32, name="pt", tag="pt")
                        eng = (och * (RPH // 2) + pr) % 2
                        for half in range(2):
                            r = pr * 2 + half
                            for ocw in range(2):
                                mm_chunk(r, och * 2 + ocw, eng=eng,
                                         dst=pt[:, half * 2 * OC + ocw * OC:
                                                half * 2 * OC + (ocw + 1) * OC])
                        nc.sync.dma_start(
                            out3[:, pr * 2:pr * 2 + 2,
                                 och * 2 * OC:(och + 1) * 2 * OC],
                            pt[:].rearrange("p (rr o) -> p rr o", rr=2),
                        )
                transpose_head_half(1)
                for r in range(RPH, NI):
                    hs_block(r)
                at_half(1)
                # phase B: tiles r8..15, full-row stores, single epi engine
                for r in range(RPH, NI):
                    ot = outbuf.tile([P, S], F32, name="ot", tag="ot")
                    for oc in range(NO):
                        mm_chunk(r, oc, ot[:, oc * OC:(oc + 1) * OC], eng=r % 2)
                    nc.sync.dma_start(out3[:, r:r + 1, :],
                                      ot[:].rearrange("p (rr o) -> p rr o", rr=1))

            if reps > 1:
                with tc.For_i(0, reps, 1):
                    body()
            else:
                body()

    nc.finalize()
    return nc


_NC_CACHE = {}


def _get_nc(reps=1):
    if reps not in _NC_CACHE:
        _NC_CACHE[reps] = build_nc(reps)
    return _NC_CACHE[reps]


def make_in_maps(head, dep, edge_U, edge_W, edge_b):
    head = np.ascontiguousarray(np.asarray(head, dtype=np.float32))
    dep = np.ascontiguousarray(np.asarray(dep, dtype=np.float32))
    u = np.asarray(edge_U, dtype=np.float32)
    w = np.asarray(edge_W, dtype=np.float32).reshape(-1)
    wh, wd = w[:D], w[D:]
    bval = float(np.asarray(edge_b).reshape(-1)[0])

    cpack = np.zeros((P, C_TOT), dtype=np.float32)
    cpack[:, C_EYE:C_EYE + P] = np.eye(P, dtype=np.float32)
    cpack[:, C_U0:C_U0 + D] = u[0:P, :]
    cpack[:, C_U1:C_U1 + D] = u[P:2 * P, :]
    cpack[:, C_WH:C_WH + D] = np.tile(wh[None, :], (P, 1))
    cpack[:, C_WDT:C_WDT + ND] = wd.reshape(ND, P).T
    cpack[:, C_B] = bval
    cpack = np.ascontiguousarray(cpack)

    return [
        {"head": head[b], "dep": dep[b], "cpack": cpack}
        for b in range(B)
    ]


def kernel(head, dep, edge_U, edge_W, edge_b):
    nc = _get_nc()
    in_maps = make_in_maps(head, dep, edge_U, edge_W, edge_b)
    res = run_bass_kernel_spmd(nc, in_maps, core_ids=list(range(B)))
    return np.stack([res.results[b]["out"] for b in range(B)], axis=0)


# revision 19
# speedup vs baseline: 1.1086x; 1.1086x over previous
"""Biaffine edge attention on 8 Trainium2 NeuronCores.

Math (per batch b):
    out[i,o] = head[i,:] @ U @ dep[o,:] + head[i,:]@wh + dep[o,:]@wd + b
with head/dep [S=2048, D=256], U [D,D], edge_W = [wh | wd] (each [D]).

Sharding: pure data-parallel over batch B=8 -> one batch per core,
U / edge_W / edge_b replicated. No collectives.

Per-core kernel:
    ATf[e,i] = sum_d U[d,e] * headT[d,i] + wd[e]      (dep-side rank-1 term
               rides the e-contraction for free)
    hs[i]    = sum_d head[i,d] * wh[d]  + b           (DVE mul+reduce)
    out[i,o] = sum_e ATf[e,i] * depT[e,o]  + hs[i]

exec_time ~= first_store_time + 16.8MB/~420GB/s + wrapup, so the whole
schedule optimizes when the first out bytes hit the store ring:
  * head is loaded in "16-consecutive-rows-per-partition" layout
    (partition p holds rows 16p..16p+15) -> 8KB contiguous DMA
    descriptors, 2 DMAs.  Out tile r then covers rows {16p + r}, an
    affine DRAM AP with 8KB-contiguous rows, and hs lines up as
    hs_col[p, r].  dep keeps the 128-row-block layout (its transpose
    must produce natural o-order for contiguous stores).
  * Load order (all triggers upfront, ACT HWDGE ring): consts, head
    half0, dep g0..g3, head half1.  Out chunk (r, oc) needs only dep
    group oc + atf(half r//8), so the first stores issue ~16us while
    dep g1..g3 are still loading.
  * Tiles r0..7 store as [128, 2x512] pair-chunks (oc-major, chasing
    dep groups); r8..15 as full [128,2048] rows.  Stores go on the sync
    (SP) HWDGE ring, independent of the load ring.
  * Transpose collect copies split DVE/ACT so neither serializes depT.
Matmuls run as float32r (1 cycle/row for moving dim >= 256).  FP32r
matmul inputs must be produced by a compute op, so matmul-feeding SBUF
tiles are float32r-typed and written by DVE/ACT copies, never by DMA.
"""

import numpy as np

import concourse.bass as bass
import concourse.tile as tile
from concourse import bacc, mybir
from concourse.bass_utils import run_bass_kernel_spmd

B, S, D = 8, 2048, 256
P = 128          # partitions
OC = 512         # matmul output free-dim chunk (one PSUM bank of fp32)
RPP = 16         # head rows per partition (out tile r covers rows 16p+r)
NI = S // P      # 16 out row-tiles
NO = S // OC     # 4 output column chunks
ND = D // P      # 2 contraction chunks
NG = 4           # dep load groups (128-row blocks x4 per group)
GB = 4           # row-blocks per dep group
HH = 2           # head halves
RPH = 8          # r-blocks per head half
F32 = mybir.dt.float32
F32R = mybir.dt.float32r

# packed const layout: eye | U0 | U1 | wh | wdT | b
C_EYE, C_U0, C_U1, C_WH, C_WDT, C_B = 0, P, P + D, P + 2 * D, P + 3 * D, P + 3 * D + ND
C_TOT = C_B + 1


def build_nc(reps=1):
    nc = bacc.Bacc("TRN2", target_bir_lowering=False, debug=False, num_devices=B)

    head_d = nc.dram_tensor("head", [S, D], F32, kind="ExternalInput")
    dep_d = nc.dram_tensor("dep", [S, D], F32, kind="ExternalInput")
    cst_d = nc.dram_tensor("cpack", [P, C_TOT], F32, kind="ExternalInput")
    out_d = nc.dram_tensor("out", [S, S], F32, kind="ExternalOutput")

    Ident = mybir.ActivationFunctionType.Identity

    with tile.TileContext(nc) as tc:
        with (
            tc.tile_pool(name="const", bufs=1) as cpool,
            tc.tile_pool(name="persist", bufs=1) as ppool,
            tc.tile_pool(name="ttrp", bufs=2) as ttrp,
            tc.tile_pool(name="pairbuf", bufs=4) as pairbuf,
            tc.tile_pool(name="outbuf", bufs=4) as outbuf,
            tc.tile_pool(name="ps_t", bufs=3, space=bass.MemorySpace.PSUM) as ps_t,
            tc.tile_pool(name="ps_mm", bufs=5, space=bass.MemorySpace.PSUM) as ps_mm,
        ):
            # ---- all loads issued upfront on the ACT HWDGE ring ----
            cst = cpool.tile([P, C_TOT], F32, name="cst", tag="cst")
            nc.scalar.dma_start(cst[:], cst_d[:])

            # trigger FIFO order on the ACT ring: cst, head h0, dep g0..g3,
            # head h1 -- head h0 early unlocks ATf for tiles r0..7; dep
            # groups next so chunk (r, oc) stores can chase them; head h1
            # last (its tiles store latest).
            head3 = head_d[0:S, :].rearrange("(p rr) d -> p rr d", rr=RPP)

            def load_head_half(hh):
                t = ppool.tile([P, RPH * D], F32, name=f"nath{hh}", tag=f"nath{hh}")
                nc.scalar.dma_start(
                    t[:].rearrange("p (r d) -> p r d", d=D),
                    head3[:, hh * RPH:(hh + 1) * RPH, :],
                )
                return t

            def load_dep_group(g):
                t = ppool.tile([P, GB * D], F32, name=f"natd{g}", tag=f"natd{g}")
                src = dep_d[g * GB * P:(g + 1) * GB * P, :]
                nc.scalar.dma_start(
                    t[:].rearrange("p (j d) -> p j d", d=D),
                    src.rearrange("(j p) d -> p j d", p=P),
                )
                return t

            nat_h = [None, None]
            nat_h[0] = load_head_half(0)
            nat_d = [load_dep_group(g) for g in range(NG)]
            nat_h[1] = load_head_half(1)

            # ---- f32r copies of U (DVE, right after cst lands) ----
            u_sb = []
            for dc in range(ND):
                u_t = cpool.tile([P, D], F32R, name=f"u{dc}", tag=f"u{dc}")
                nc.vector.tensor_copy(u_t[:], cst[:, C_U0 + dc * D:C_U0 + (dc + 1) * D])
                u_sb.append(u_t)
            eye = cst[:, C_EYE:C_EYE + P]

            # ---- persistent SBUF tensors ----
            headT = [ppool.tile([P, S], F32R, name=f"headT{dc}", tag=f"headT{dc}")
                     for dc in range(ND)]
            depT = [ppool.tile([P, S], F32R, name=f"depT{dc}", tag=f"depT{dc}")
                    for dc in range(ND)]
            atf = [ppool.tile([P, S], F32R, name=f"atf{eb}", tag=f"atf{eb}")
                   for eb in range(ND)]
            hs_colb = ppool.tile([P, NI], F32, name="hs_colb", tag="hs_colb")

            out3 = out_d[0:S, :].rearrange("(p rr) o -> p rr o", rr=RPP)

            def transpose_head_quad(hh, q):
                # 8 PE transposes -> 2 [128,512] collect tiles (DVE + ACT).
                # headT columns land as (r, p), r = hh*8 + rloc.
                nat = nat_h[hh]
                for dc in range(ND):
                    pst = ps_t.tile([P, 4 * P], F32, name="pst", tag="pst")
                    for k in range(4):
                        rloc = q * 4 + k
                        nc.tensor.transpose(
                            pst[:, k * P:(k + 1) * P],
                            nat[:, rloc * D + dc * P: rloc * D + dc * P + P],
                            eye,
                        )
                    dst = headT[dc][:, (hh * RPH + q * 4) * P:
                                     (hh * RPH + q * 4 + 4) * P]
                    if dc == 0:
                        nc.vector.tensor_copy(dst, pst[:])
                    else:
                        nc.scalar.copy(dst, pst[:])

            def transpose_dep_group(g):
                # g0/g1 collects split DVE/ACT (critical path to the first
                # stores); g2/g3 go to ACT which has slack by then.
                nat = nat_d[g]
                for dc in range(ND):
                    pst = ps_t.tile([P, GB * P], F32, name="pst", tag="pst")
                    for j in range(GB):
                        nc.tensor.transpose(
                            pst[:, j * P:(j + 1) * P],
                            nat[:, j * D + dc * P: j * D + dc * P + P],
                            eye,
                        )
                    dst = depT[dc][:, g * GB * P:(g + 1) * GB * P]
                    if g < 2 and dc == 0:
                        nc.vector.tensor_copy(dst, pst[:])
                    else:
                        nc.scalar.copy(dst, pst[:])

            def hs_block(r):
                # hs_colb[p, r] = b + sum_d nat_h[p, r*D+d] * wh[d] -- the
                # mul runs on the otherwise-idle GpSimd; only the cheap
                # X-reduce (+bias) lands on DVE.
                hh, rloc = r // RPH, r % RPH
                ttr = ttrp.tile([P, D], F32, name="ttr", tag="ttr")
                nc.gpsimd.tensor_mul(
                    ttr[:], nat_h[hh][:, rloc * D:(rloc + 1) * D],
                    cst[:, C_WH:C_WH + D],
                )
                hsr = ttrp.tile([P, 1], F32, name="hsr", tag="hsr")
                nc.vector.reduce_sum(hsr[:], ttr[:], axis=mybir.AxisListType.X)
                nc.vector.tensor_scalar_add(
                    hs_colb[:, r:r + 1], hsr[:], cst[:, C_B:C_B + 1],
                )

            def at_quad(hh, q):
                # ATf for one 512-i-column quad; eb0 copy on ACT, eb1 on DVE
                c0 = (hh * 2 + q) * OC
                for eb in range(ND):
                    pa = ps_mm.tile([P, OC], F32, name="psmm", tag="psmm")
                    for dc in range(ND):
                        nc.tensor.matmul(
                            pa[:],
                            u_sb[dc][:, eb * P:(eb + 1) * P],
                            headT[dc][:, c0:c0 + OC],
                            start=(dc == 0),
                            stop=(dc == ND - 1),
                        )
                    wdb = cst[:, C_WDT + eb:C_WDT + eb + 1]
                    if eb == 0:
                        nc.scalar.activation(
                            atf[eb][:, c0:c0 + OC], pa[:], Ident, bias=wdb)
                    else:
                        nc.vector.tensor_scalar_add(
                            atf[eb][:, c0:c0 + OC], pa[:], wdb)

            def mm_chunk(r, oc, dst, eng):
                po = ps_mm.tile([P, OC], F32, name="psmm", tag="psmm")
                for eb in range(ND):
                    nc.tensor.matmul(
                        po[:],
                        atf[eb][:, r * P:(r + 1) * P],
                        depT[eb][:, oc * OC:(oc + 1) * OC],
                        start=(eb == 0),
                        stop=(eb == ND - 1),
                    )
                if eng == 0:
                    nc.scalar.activation(dst, po[:], Ident, bias=hs_colb[:, r:r + 1])
                else:
                    nc.vector.tensor_scalar_add(dst, po[:], hs_colb[:, r:r + 1])

            def pair_store(pr, och, eng):
                # [128, 2 rows x 1024 cols] mega-chunk store (4KB descriptors)
                pt = pairbuf.tile([P, 2 * 2 * OC], F32, name="pt", tag="pt")
                for half in range(2):
                    r = pr * 2 + half
                    for ocw in range(2):
                        mm_chunk(r, och * 2 + ocw, eng=eng,
                                 dst=pt[:, half * 2 * OC + ocw * OC:
                                        half * 2 * OC + (ocw + 1) * OC])
                nc.sync.dma_start(
                    out3[:, pr * 2:pr * 2 + 2,
                         och * 2 * OC:(och + 1) * 2 * OC],
                    pt[:].rearrange("p (rr o) -> p rr o", rr=2),
                )

            def body():
                # emission order == scheduler priority: the exact critical
                # chain to the first store goes first, distractor work last.
                transpose_head_quad(0, 0)          # cols for r0..3
                at_quad(0, 0)
                transpose_dep_group(0)
                transpose_dep_group(1)
                for r in range(4):
                    hs_block(r)
                pair_store(0, 0, eng=0)            # r0,r1 x oc0,oc1
                pair_store(1, 0, eng=1)            # r2,r3
                transpose_head_quad(0, 1)          # cols for r4..7
                at_quad(0, 1)
                for r in range(4, RPH):
                    hs_block(r)
                pair_store(2, 0, eng=0)
                pair_store(3, 0, eng=1)
                transpose_dep_group(2)
                transpose_dep_group(3)
                for pr in range(RPH // 2):
                    pair_store(pr, 1, eng=pr % 2)
                transpose_head_quad(1, 0)
                at_quad(1, 0)
                transpose_head_quad(1, 1)
                at_quad(1, 1)
                for r in range(RPH, NI):
                    hs_block(r)
                # phase B: tiles r8..15, full-row stores, single epi engine
                for r in range(RPH, NI):
                    ot = outbuf.tile([P, S], F32, name="ot", tag="ot")
                    for oc in range(NO):
                        mm_chunk(r, oc, ot[:, oc * OC:(oc + 1) * OC], eng=r % 2)
                    nc.sync.dma_start(out3[:, r:r + 1, :],
                                      ot[:].rearrange("p (rr o) -> p rr o", rr=1))

            if reps > 1:
                with tc.For_i(0, reps, 1):
                    body()
            else:
                body()

    nc.finalize()
    return nc


_NC_CACHE = {}


def _get_nc(reps=1):
    if reps not in _NC_CACHE:
        _NC_CACHE[reps] = build_nc(reps)
    return _NC_CACHE[reps]


def make_in_maps(head, dep, edge_U, edge_W, edge_b):
    head = np.ascontiguousarray(np.asarray(head, dtype=np.float32))
    dep = np.ascontiguousarray(np.asarray(dep, dtype=np.float32))
    u = np.asarray(edge_U, dtype=np.float32)
    w = np.asarray(edge_W, dtype=np.float32).reshape(-1)
    wh, wd = w[:D], w[D:]
    bval = float(np.asarray(edge_b).reshape(-1)[0])

    cpack = np.zeros((P, C_TOT), dtype=np.float32)
    cpack[:, C_EYE:C_EYE + P] = np.eye(P, dtype=np.float32)
    cpack[:, C_U0:C_U0 + D] = u[0:P, :]
    cpack[:, C_U1:C_U1 + D] = u[P:2 * P, :]
    cpack[:, C_WH:C_WH + D] = np.tile(wh[None, :], (P, 1))
    cpack[:, C_WDT:C_WDT + ND] = wd.reshape(ND, P).T
    cpack[:, C_B] = bval
    cpack = np.ascontiguousarray(cpack)

    return [
        {"head": head[b], "dep": dep[b], "cpack": cpack}
        for b in range(B)
    ]


def kernel(head, dep, edge_U, edge_W, edge_b):
    nc = _get_nc()
    in_maps = make_in_maps(head, dep, edge_U, edge_W, edge_b)
    res = run_bass_kernel_spmd(nc, in_maps, core_ids=list(range(B)))
    return np.stack([res.results[b]["out"] for b in range(B)], axis=0)
